# revision 30
# baseline (speedup 1.0000x reference)
"""Multi-head causal self-attention with RoPE on 8 Trainium2 NeuronCores.

Problem: x[2,2048,2048], wq/wk/wv/wo[2048,2048] fp32, 16 heads (hd=128),
interleaved RoPE, causal softmax, Megatron-style tensor parallelism over
heads: 2 heads per core, wo row-sharded, partial outputs summed on host.

All matmul operands are bf16 (measured end-to-end rel err ~3e-3 vs the
2e-2 gate); PSUM accumulation is fp32. bf16 stationaries get the
compiler's fast-weight-load path so LDWEIGHTS hides under the moving
stream, and all DMA volumes halve vs fp32.

Layout strategy (per core, per batch b):
  - host supplies xT = x^T [d, s] (bf16) and weight slices pre-transposed
  - projections: qT,kT per head via lhsT=w-tile [d,e], rhs=xT [d,s512]
    -> q^T,k^T [e=128, s] directly; v natural [s, e] via lhsT=xT-subtile;
    RoPE fused right after each projection chunk (rot matmul + DVE).
  - attention per (b, j-block of 512 q), heads interleaved and scores
    pipelined two kv-tiles ahead so the exp (ACT) latency never stalls
    the PE:
      scoresT[kv=128, q<=512] = kT-tile.T @ qT-block
      attn = exp(scoresT) on ACT (bf16); triangle mask on 128-col band
      oT[d, q] += v-tile.T @ attn ; rowsum[128, q] += ones.T @ attn
      oT_norm = oT * reciprocal_approx_fast(rowsum) (DVE, bf16)
  - output projection per j-block: yT[e, jsl] = sum_ct woT-tile.T @ oT,
    copied PSUM->SBUF on DVE, DMA'd per 128-row tile -> DRAM f32
  - host: y = sum over cores of yT^T
"""

import os
import sys

for _p in ("/opt/trn_rl_repo", "/root/.axon_site/_ro/trn_rl_repo"):
    if os.path.isdir(_p) and _p not in sys.path:
        sys.path.append(_p)

import numpy as np

import concourse.bacc as bacc
import concourse.mybir as mybir
import concourse.tile as tile
from concourse.alu_op_type import AluOpType
from concourse.bass_utils import run_bass_kernel_spmd

F32 = mybir.dt.float32
BF16 = mybir.dt.bfloat16

B, S, D = 2, 2048, 2048
H, HD = 16, 128
NCORES = 8
HPC = H // NCORES            # heads per core = 2
CPC = HPC * HD               # channels per core = 256
P = 128
SC = 512                     # s-chunk for projections / q-block for attention
NSC = S // SC                # 4
NDT = D // P                 # 16 contraction tiles
NG = 2                       # x-tile DMA group: d-tiles per DMA
ROPE_THETA = 10000.0

Exp = mybir.ActivationFunctionType.Exp

last_exec_time_ns = None
_nc_cache = None


def _build_nc():
    nc = bacc.Bacc("TRN2", target_bir_lowering=False, debug=False)

    xT = nc.dram_tensor("xT", [B, D, S], BF16, kind="ExternalInput")
    wqkvT = nc.dram_tensor("wqkvT", [D, 6 * P], BF16, kind="ExternalInput")
    woT = nc.dram_tensor("woT", [CPC, D], BF16, kind="ExternalInput")
    cosT = nc.dram_tensor("cosT", [HD, S], F32, kind="ExternalInput")
    sinT = nc.dram_tensor("sinT", [HD, S], F32, kind="ExternalInput")
    rotL = nc.dram_tensor("rotL", [HD, HD], BF16, kind="ExternalInput")
    trimask = nc.dram_tensor("trimask", [P, P], BF16, kind="ExternalInput")
    ones = nc.dram_tensor("ones", [P, P], BF16, kind="ExternalInput")
    yT = nc.dram_tensor("yT", [B, D, S], BF16, kind="ExternalOutput")

    xTr = xT.rearrange("b (o p) s -> b p o s", p=P)
    wqr = wqkvT.rearrange("(o p) e -> p o e", p=P)

    with tile.TileContext(nc) as tc:
        with tc.tile_pool(name="const", bufs=1) as constp, \
             tc.tile_pool(name="xp", bufs=16) as xp, \
             tc.tile_pool(name="qk", bufs=2) as qkp, \
             tc.tile_pool(name="vp", bufs=2) as vp, \
             tc.tile_pool(name="op", bufs=2) as op_, \
             tc.tile_pool(name="attn", bufs=10) as attnp, \
             tc.tile_pool(name="asum", bufs=3) as sump, \
             tc.tile_pool(name="tmp", bufs=4) as tmpp, \
             tc.tile_pool(name="yt", bufs=8) as ytp, \
             tc.tile_pool(name="ps", bufs=4, space="PSUM") as psp, \
             tc.tile_pool(name="acc", bufs=4, space="PSUM") as accp:

            wq_sb = constp.tile([P, NDT, 6 * P], BF16)
            wo_sb = constp.tile([P, CPC // P, D], BF16)
            cos_sb = constp.tile([P, S], F32)
            sin_sb = constp.tile([P, S], F32)
            rot_sb = constp.tile([P, P], BF16)
            mask_sb = constp.tile([P, P], BF16)
            ones_sb = constp.tile([P, P], BF16)

            # ---- x chunk DMAs: 8 group-tiles per (b, sc) chunk; two chunks
            #      live in SBUF, prefetch runs one chunk ahead. Steady-state
            #      prefetch uses only the gpsimd/sync queues — a DMA on the
            #      scalar queue would delay attention exps ----
            chunk_tiles = {}

            def chunk_dma(c, queues=(nc.gpsimd, nc.sync, nc.scalar)):
                b, sc = divmod(c, NSC)
                xts = []
                for g in range(NDT // NG):
                    xt = xp.tile([P, NG, SC], BF16, tag="xt")
                    queues[g % len(queues)].dma_start(
                        xt[:], xTr[b, :, g * NG:(g + 1) * NG,
                                   sc * SC:(sc + 1) * SC])
                    xts.append(xt)
                chunk_tiles[c] = xts

            # startup: interleave weight d-tiles and chunk-0 x groups in
            # consumption order, round-robin over all three DMA queues, so
            # the first accumulation is fed as fast as possible
            startup = []
            for g in range(NDT // NG):
                startup.append(("x", g))
                startup.append(("w", 2 * g))
                startup.append(("w", 2 * g + 1))
            c0_tiles = []
            rr = [nc.sync, nc.scalar, nc.gpsimd]
            for i, (kind, idx) in enumerate(startup):
                q = rr[i % 3]
                if kind == "w":
                    # qk columns only — the v columns aren't needed until
                    # the first chunk's e-loop is done
                    q.dma_start(wq_sb[:, idx, 0:4 * P], wqr[:, idx, 0:4 * P])
                else:
                    xt = xp.tile([P, NG, SC], BF16, tag="xt", name=f"xt0_{idx}")
                    q.dma_start(
                        xt[:], xTr[0, :, idx * NG:(idx + 1) * NG, 0:SC])
                    c0_tiles.append(xt)
            chunk_tiles[0] = c0_tiles
            nc.sync.dma_start(rot_sb[:], rotL[:])
            for dt in range(NDT):
                rr[dt % 3].dma_start(wq_sb[:, dt, 4 * P:6 * P],
                                     wqr[:, dt, 4 * P:6 * P])
            chunk_dma(1, queues=(nc.gpsimd, nc.sync, nc.scalar))
            nc.sync.dma_start(cos_sb[:], cosT[:])
            nc.sync.dma_start(sin_sb[:], sinT[:])
            nc.sync.dma_start(mask_sb[:], trimask[:])
            nc.sync.dma_start(ones_sb[:], ones[:])
            nc.sync.dma_start(wo_sb[:], woT.rearrange("(o p) e -> p o e", p=P))

            qkT = {}
            v_sb = {}

            def emit_rope(b, e, sl):
                pr = psp.tile([P, SC], F32, tag="ps")
                nc.tensor.matmul(pr[:], rot_sb[:], qkT[b, e][:, sl],
                                 start=True, stop=True)
                tmp = tmpp.tile([P, SC], BF16, tag="ropetmp")
                nc.vector.tensor_tensor(tmp[:], pr[:], sin_sb[:, sl],
                                        AluOpType.mult)
                nc.vector.tensor_tensor(qkT[b, e][:, sl], qkT[b, e][:, sl],
                                        cos_sb[:, sl], AluOpType.mult)
                nc.vector.tensor_tensor(qkT[b, e][:, sl], qkT[b, e][:, sl],
                                        tmp[:], AluOpType.add)

            def emit_proj_chunk(b, sc, flush_pending):
                c = b * NSC + sc
                xts = chunk_tiles.pop(c)
                sl = slice(sc * SC, (sc + 1) * SC)
                # dt-outer / e-inner: each arriving (x, wq) d-tile unlocks
                # four matmuls, so the PE keeps pace with the DMA stream
                # while the first chunks are still landing
                pq = [psp.tile([P, SC], F32, tag="ps", name=f"pq{e}")
                      for e in range(4)]
                for dt in range(NDT):
                    for e in range(4):
                        nc.tensor.matmul(pq[e][:],
                                         wq_sb[:, dt, e * P:(e + 1) * P],
                                         xts[dt // NG][:, dt % NG, :],
                                         start=(dt == 0), stop=(dt == NDT - 1),
                                         skip_group_check=True)
                for e in range(4):
                    nc.scalar.copy(qkT[b, e][:, sl], pq[e][:])
                if flush_pending is not None:
                    flush_pending()
                for ss in range(SC // P):
                    pv = psp.tile([P, SC], F32, tag="ps")
                    pvv = pv[:, :CPC]
                    for dt in range(NDT):
                        nc.tensor.matmul(pvv,
                                         xts[dt // NG][:, dt % NG,
                                                       ss * P:(ss + 1) * P],
                                         wq_sb[:, dt, 4 * P:6 * P],
                                         start=(dt == 0), stop=(dt == NDT - 1))
                    # rope for channel-block ss rides behind this v block,
                    # giving its ACT copy time to complete
                    emit_rope(b, ss, sl)
                    nc.scalar.copy(v_sb[b][:, sc * (SC // P) + ss, :], pvv)
                # prefetch chunk c+2 only after chunk c's readers are emitted,
                # so the buffer-reuse WAR dependency is tracked
                if c + 2 < B * NSC:
                    chunk_dma(c + 2)

            pending_oproj = [None]

            def flush_oproj():
                fn = pending_oproj[0]
                if fn is not None:
                    pending_oproj[0] = None
                    fn()

            oT = {}
            for b in range(B):
                for e in range(4):
                    qkT[b, e] = qkp.tile([P, S], BF16, tag=f"qk{e}",
                                         name=f"qkT{b}_{e}")
                v_sb[b] = vp.tile([P, NDT, CPC], BF16, tag="v", name=f"v{b}")
                for sc in range(NSC):
                    emit_proj_chunk(b, sc,
                                    flush_oproj if sc == 0 else None)

                # ---- attention: j outer, heads interleaved, scores three
                #      kv-tiles ahead so exp latency is hidden; each j's
                #      output projection is deferred into the next j-block's
                #      (or batch's) pipeline so the PE never waits on the
                #      DVE softmax-normalization chain ----
                # per-head oT tiles: the output projection's first (ct=0)
                # matmul then depends only on head 0's normalization
                oT[b] = [op_.tile([P, S], BF16, tag=f"o{h}", name=f"oT{b}_{h}")
                         for h in range(HPC)]

                def emit_score(j, t, h, b=b):
                    dp = t - (SC // P) * j
                    dlt = max(dp, 0) * P
                    qsl = slice(j * SC + dlt, (j + 1) * SC)
                    pscore = psp.tile([P, SC], F32, tag="ps")
                    nc.tensor.matmul(pscore[:, dlt:],
                                     qkT[b, 2 + h][:, t * P:(t + 1) * P],
                                     qkT[b, h][:, qsl],
                                     start=True, stop=True)
                    at = attnp.tile([P, SC], BF16, tag="attn")
                    nc.scalar.activation(at[:, dlt:], pscore[:, dlt:],
                                         Exp, bias=0.0, scale=1.0)
                    if dp >= 0:  # triangle mask on the 128-col band
                        nc.vector.tensor_tensor(
                            at[:, dlt:dlt + P], at[:, dlt:dlt + P],
                            mask_sb[:], AluOpType.mult)
                    return at

                def make_oproj(j, b=b, in_proj=False, final=False):
                    jsl = slice(j * SC, (j + 1) * SC)

                    def emit():
                        # PSUM->SBUF copies: all-DVE when this flush lands in
                        # an attention window (ACT is exp-saturated there);
                        # DVE/ACT alternating in projection windows. Final
                        # block's stores use all three DMA queues.
                        store_q = ([nc.sync, nc.gpsimd, nc.scalar] if final
                                   else [nc.sync, nc.gpsimd])
                        for et in range(NDT):
                            py = psp.tile([P, SC], F32, tag="ps")
                            for ct in range(HPC):
                                nc.tensor.matmul(
                                    py[:],
                                    wo_sb[:, ct, et * P:(et + 1) * P],
                                    oT[b][ct][:, jsl],
                                    start=(ct == 0), stop=(ct == HPC - 1))
                            yt = ytp.tile([P, SC], BF16, tag="yt")
                            if in_proj and et % 2 == 1:
                                nc.scalar.copy(yt[:], py[:])
                            else:
                                nc.vector.tensor_copy(yt[:], py[:])
                            store_q[et % len(store_q)].dma_start(
                                yT[b, et * P:(et + 1) * P, jsl], yt[:])
                    return emit

                LA = 3  # score lookahead in kv-tiles
                att_q = {}  # (t, h) -> attn tile, pipelined
                at_prev = {}  # stashed even-t full tiles awaiting pair-sum
                for tp in range(LA):
                    for h in range(HPC):
                        att_q[tp, h] = emit_score(0, tp, h)

                for j in range(NSC):
                    jsl = slice(j * SC, (j + 1) * SC)
                    n_kv = (SC // P) * (j + 1)
                    po = [accp.tile([P, SC], F32, tag="acc", name=f"po{h}")
                          for h in range(HPC)]
                    prs = [accp.tile([P, SC], F32, tag="acc", name=f"prs{h}")
                           for h in range(HPC)]
                    for t in range(n_kv):
                        nxt = t + LA
                        if nxt < n_kv:
                            for h in range(HPC):
                                att_q[nxt, h] = emit_score(j, nxt, h)
                        elif j + 1 < NSC:
                            for h in range(HPC):
                                att_q[nxt - n_kv, h] = emit_score(
                                    j + 1, nxt - n_kv, h)
                        dlt = max(t - (SC // P) * j, 0) * P
                        full = t < (SC // P) * j
                        for h in range(HPC):
                            at = att_q.pop((t, h))
                            nc.tensor.matmul(po[h][:, dlt:],
                                             v_sb[b][:, t, h * HD:(h + 1) * HD],
                                             at[:, dlt:],
                                             start=(t == 0), stop=(t == n_kv - 1),
                                             skip_group_check=True)
                            if full and t % 2 == 0:
                                at_prev[h] = at
                            elif full:
                                # sum the pair of full attn tiles on DVE, then
                                # one rowsum matmul streams half the elements
                                s2 = sump.tile([P, SC], BF16, tag="asum",
                                               name=f"asum{h}")
                                nc.vector.tensor_tensor(
                                    s2[:], at_prev.pop(h)[:], at[:],
                                    AluOpType.add)
                                nc.tensor.matmul(prs[h][:], ones_sb[:], s2[:],
                                                 start=(t == 1), stop=False,
                                                 skip_group_check=True)
                            else:
                                nc.tensor.matmul(prs[h][:, dlt:], ones_sb[:],
                                                 at[:, dlt:],
                                                 start=(t == 0),
                                                 stop=(t == n_kv - 1),
                                                 skip_group_check=True)
                        if t == 1:
                            flush_oproj()
                    for h in range(HPC):
                        recip = tmpp.tile([P, SC], F32, tag="recip")
                        nc.vector.reciprocal_approx_fast(recip[:], prs[h][:])
                        nc.vector.tensor_tensor(oT[b][h][:, jsl], po[h][:],
                                                recip[:], AluOpType.mult)
                    pending_oproj[0] = make_oproj(
                        j, in_proj=(j == NSC - 1),
                        final=(b == B - 1 and j == NSC - 1))
            flush_oproj()
    nc.finalize()
    return nc


def _host_inputs(x, wq, wk, wv, wo):
    """Build per-core input maps (host-side shard + transform)."""
    import ml_dtypes
    bf16 = ml_dtypes.bfloat16
    scale = 1.0 / np.sqrt(np.float32(HD))

    xTr = np.ascontiguousarray(x.transpose(0, 2, 1)).astype(bf16)

    # RoPE tables in [e, s] layout (same for every head)
    inv_freq = 1.0 / (ROPE_THETA ** (np.arange(0, HD, 2, dtype=np.float64) / HD))
    ang = np.arange(S, dtype=np.float64)[None, :] * inv_freq[:, None]  # [64, S]
    cosT = np.repeat(np.cos(ang), 2, axis=0).astype(np.float32)  # [128, S]
    sinT = np.repeat(np.sin(ang), 2, axis=0).astype(np.float32)

    # signed pair-swap: qrot[2i] = -q[2i+1], qrot[2i+1] = q[2i]
    # matmul computes qrot[m, s] = sum_k rotL[k, m] q[k, s]
    rotL = np.zeros((HD, HD), dtype=np.float32)
    for i in range(HD // 2):
        rotL[2 * i + 1, 2 * i] = -1.0
        rotL[2 * i, 2 * i + 1] = 1.0
    rotL = rotL.astype(bf16)

    r = np.arange(P)[:, None]
    c = np.arange(P)[None, :]
    trimask = (c >= r).astype(bf16)  # [128,128] upper-right valid

    wq_s = (wq * scale).astype(bf16)
    wk_s = wk.astype(bf16)
    wv_s = wv.astype(bf16)
    wo_s = wo.astype(bf16)

    in_maps = []
    for cix in range(NCORES):
        rows = slice(cix * CPC, (cix + 1) * CPC)  # head-channel rows
        blocks = []
        for h in range(HPC):
            hr = slice((cix * HPC + h) * HD, (cix * HPC + h + 1) * HD)
            blocks.append(wq_s[hr])   # q_h: [128, D]
        for h in range(HPC):
            hr = slice((cix * HPC + h) * HD, (cix * HPC + h + 1) * HD)
            blocks.append(wk_s[hr])
        blocks.append(wv_s[rows])     # v both heads: [256, D]
        wqkvT = np.ascontiguousarray(
            np.concatenate(blocks, axis=0).T)  # [D, 768] bf16
        woT = np.ascontiguousarray(wo_s[:, rows].T)  # [256, D] bf16
        in_maps.append({
            "xT": xTr,
            "wqkvT": wqkvT,
            "woT": woT,
            "cosT": cosT,
            "sinT": sinT,
            "rotL": rotL,
            "trimask": trimask,
            "ones": np.ones((P, P), dtype=bf16),
        })
    return in_maps


def _get_nc():
    global _nc_cache
    if _nc_cache is None:
        _nc_cache = _build_nc()
    return _nc_cache


def kernel(x, wq, wk, wv, wo, _trace=False):
    global last_exec_time_ns
    nc = _get_nc()
    in_maps = _host_inputs(np.asarray(x, dtype=np.float32),
                           np.asarray(wq, dtype=np.float32),
                           np.asarray(wk, dtype=np.float32),
                           np.asarray(wv, dtype=np.float32),
                           np.asarray(wo, dtype=np.float32))
    res = run_bass_kernel_spmd(nc, in_maps, core_ids=list(range(NCORES)),
                               trace=_trace)
    last_exec_time_ns = res.exec_time_ns
    y = np.zeros((B, S, D), dtype=np.float64)
    for cix in range(NCORES):
        y += res.results[cix]["yT"].transpose(0, 2, 1).astype(np.float64)
    return y.astype(np.float32)
